# revision 110
# baseline (speedup 1.0000x reference)
"""Multi-head causal attention (B=2, S=2048, D=1024, H=16) on 8 trn2 cores.

Sharding: tensor-parallel over heads. Each core owns 2 heads: a 128-column
slice of w_q/w_k/w_v and the matching 128-row slice of w_o. Every core
computes a full [B*S, D] partial output in bf16; the host sums the 8 partials
in f32 and adds the bias.

FP8 DoubleRow matmuls (0.5 PE cycles/row, two 128-deep k-tiles contracted
per instruction) carry the projections and the score matmuls; the AV and
out-projection matmuls stay bf16 (1 cycle/row) because attention-weight /
ctx quantization to fp8 would blow the 2e-2 error budget. Precision is held
by hi/lo fp8 decompositions:

  - x is shipped as xh = fp8(x) plus xl = fp8(16*(x - xh)); weights are
    pre-scaled by 32 on the host and split into wh = fp8(32W),
    whd16 = fp8(wh/16) and (v only) wl = fp8(32W - wh). Q/K projections run
    1 term (xh@wh: the score-side fp8 quantization already dominates that
    path); the V projection runs 3 terms (xh@wh + xl@whd16 + xh@wl), making
    v effectively bf16-exact since v feeds the output linearly.
  - Scores: q,k evicted straight from PSUM to fp8 (the score quantization)
    into a zero-padded pair layout [128p, 2, BS] (pair slot 1 is zeros, so a
    K=64 DoubleRow contraction per head computes an exact fp8 dot). The exp
    activation scale absorbs the 32*32 weight prescale (2^-13).
  - The w_o slice is pre-divided by 32 on the host so the 32x-scaled ctx
    cancels; the softmax denominator ratio is scale-invariant.

Measured on-device absmax rel err 1.37e-2 (gate 2e-2).

The per-core schedule is one software-pipelined PE instruction stream over
8 query chunks of 512 rows:

  - Chunk sc's score/exp/AV tiles are woven with "filler" matmuls: the
    out-projection from two chunks back (giving the normalize chain a full
    chunk of slack) plus chunk sc+1's projections and V transposes, so
    nothing in a chunk waits on its own projections. Fillers hide the ~1us
    ACT exp latency per score tile and keep the PE at the full 2.4GHz
    p-state (any >100ns gap halves the PE clock for 3us).
  - Causal masking rides the score PSUM: an ident @ umask matmul adds
    -30*2^13 above the diagonal of each boundary block before exp, so the
    masked weights round to ~1e-13 and the AV path has no Pool dependency.
  - V is transposed [seq, hd] by PE transposes into `vone`, whose
    ones-columns (at 128/129 of a 64B-aligned 160-elem stride) make the AV
    matmul accumulate the softmax denominator for free; host-side
    interleaving of w_v's columns makes the transposed layout match the
    strided (h::2) stationary reads directly. All 4 transposes of a chunk
    share one PSUM tile / accumulation group (start on the first slice
    only: a later start=True would re-mark the shared 2KB zero-region) and
    evict with a single batched DVE copy.
  - Engine placement: PSUM can only be read by DVE/ACT (gpsimd cannot), so
    DVE carries the eviction traffic, ACT carries exp, and Pool gets the
    all-SBUF work (normalize muls, broadcasts, memsets).
  - Chunk normalize is split: the ctx PSUM eviction lands at chunk end
    (freeing the single ctx-PSUM slot before the filler flush), the
    recip/broadcast/mul finish runs early next chunk, and oproj consumes
    ctxT one chunk later still.
  - PSUM budget (8 banks): 2 proj/oproj+transpose ring, 4 score pair ring,
    2 ctx accumulator.
  - The last chunk accumulates ctx column-halves into separate PSUM banks
    ([65, half, head, 256]) so the left half normalizes under the final
    AVs; the right half's chain (PSUM-direct reciprocals in parallel with
    DVE+ACT evictions) runs under banked fillers, and the out-proj
    quarters chase the muls with per-st-tile stores, the last one split
    across two DMA rings.
  - DMA: wqh loads first on the SP ring (the scalar ring's sequencer is
    blocked ~1.3us by the activation-table load), then the first x chunk,
    then the remaining packed-weight pieces.

Cost-model timeline: 113.3us per core (bf16 predecessor: 147.1us).
"""

import sys

sys.path.insert(0, "/opt/trn_rl_repo")

import numpy as np
import ml_dtypes

import concourse.bass as bass
import concourse.mybir as mybir
import concourse.tile as tile
from concourse import bacc
from concourse.bass_utils import run_bass_kernel_spmd

B, S, D, H, HD = 2, 2048, 1024, 16, 64
BS = B * S                  # 4096 flattened rows
NCORES = 8
DC = D // NCORES            # 128 head-dims per core (2 heads)
P = 128                     # partitions
SC = 512                    # s-chunk (moving free dim)
NSC = BS // SC              # 8 s-chunks over the flattened rows
NKT = D // P                # 8 k-tiles for the projections
NQC = S // SC               # 4 q-chunks per batch
NST = BS // P               # 32 s-tiles of 128
SPB = S // P                # 16 s-tiles per batch

F32 = mybir.dt.float32
BF16 = mybir.dt.bfloat16
F8 = mybir.dt.float8e4
DR = mybir.MatmulPerfMode.DoubleRow
DEBUG_DUMP = False
EXP = mybir.ActivationFunctionType.Exp
EXP_SCALE = 0.125 / 1024.0  # 2^-13: 1/sqrt(64) plus the 32*32 weight prescale
QK_TERMS = 1  # hi/lo terms in the q/k projections (1, 2 or 3); 1-term adds
              # only ~1e-3 absmax error (score-side fp8 quantization already
              # dominates the q/k path) and saves 16k PE cycles

LABELS = {}


def _lbl(bi, label):
    try:
        LABELS[bi.ins.name] = label
    except Exception:
        pass
    return bi


def _rr(*groups):
    """Round-robin interleave lists (preserving each list's order)."""
    out = []
    idx = [0] * len(groups)
    while True:
        progressed = False
        for gi, g in enumerate(groups):
            if idx[gi] < len(g):
                out.append(g[idx[gi]])
                idx[gi] += 1
                progressed = True
        if not progressed:
            return out


def _build_nc():
    nc = bacc.Bacc(None, target_bir_lowering=False)

    NW8 = {1: 5, 2: 7, 3: 9}[QK_TERMS]
    xh = nc.dram_tensor("xh", [D, BS], F8, kind="ExternalInput")
    xl = nc.dram_tensor("xl", [D, BS], F8, kind="ExternalInput")
    # all fp8 weight splits packed into one tensor; slot order depends on
    # QK_TERMS (see _w8_slots)
    w8 = nc.dram_tensor("w8", [P, NW8, NKT, DC], F8, kind="ExternalInput")
    wo = nc.dram_tensor("wo", [DC, D], BF16, kind="ExternalInput")
    ident = nc.dram_tensor("ident", [P, P], BF16, kind="ExternalInput")
    # causal mask constant: -245760 (= -30 * 2^13, exp-scale units) above the
    # diagonal, 0 elsewhere; added into the score PSUM via an ident matmul.
    umask = nc.dram_tensor("umask", [P, P], BF16, kind="ExternalInput")
    out = nc.dram_tensor("out", [BS, D], BF16, kind="ExternalOutput")
    if DEBUG_DUMP:
        dbg_vone = nc.dram_tensor("dbg_vone", [P, NST, 160], BF16, kind="ExternalOutput")
        dbg_qt = nc.dram_tensor("dbg_qt", [P, 2, BS], F8, kind="ExternalOutput")
        dbg_kt = nc.dram_tensor("dbg_kt", [P, 2, BS], F8, kind="ExternalOutput")
        dbg_ctxT = nc.dram_tensor("dbg_ctxT", [P, BS], BF16, kind="ExternalOutput")

    with tile.TileContext(nc) as tc:
        with (
            tc.tile_pool(name="big", bufs=1) as big,
            tc.tile_pool(name="xts", bufs=2) as xts,
            tc.tile_pool(name="ob", bufs=4) as obs,
            tc.tile_pool(name="et", bufs=6) as etp,
            tc.tile_pool(name="small", bufs=4) as small,
            tc.tile_pool(name="ps_p", bufs=2, space="PSUM") as ps_p,   # proj + oproj [128,512]
            tc.tile_pool(name="ps_s", bufs=2, space="PSUM") as ps_sp,  # score pairs [128,2,512]
            tc.tile_pool(name="ps_c", bufs=1, space="PSUM") as ps_cp,  # ctx pair [65,2,512]
        ):
            # fp8 q/k in zero-padded pair layout: slot 0 carries the data,
            # slot 1 stays zero so a K=64 DoubleRow matmul contracts exactly.
            qt = big.tile([P, 2, BS], F8, tag="qt")
            kt = big.tile([P, 2, BS], F8, tag="kt")
            vt = big.tile([P, BS], BF16, tag="vt")
            ctxT = big.tile([P, BS], BF16, tag="ctxT")
            vone = big.tile([P, NST, 160], BF16, tag="vone")
            w8_sb = big.tile([P, NW8, NKT, DC], F8, tag="w8")
            # weight-slot (slot, which_x) term lists per projection
            if QK_TERMS == 1:
                QTERMS, KTERMS = [(0, 0)], [(1, 0)]
                VTERMS = [(2, 0), (3, 1), (4, 0)]
            elif QK_TERMS == 2:
                QTERMS, KTERMS = [(0, 0), (1, 1)], [(2, 0), (3, 1)]
                VTERMS = [(4, 0), (5, 1), (6, 0)]
            else:
                QTERMS = [(0, 0), (1, 1), (7, 0)]
                KTERMS = [(2, 0), (3, 1), (8, 0)]
                VTERMS = [(4, 0), (5, 1), (6, 0)]
            NQW = len(QTERMS) * 2  # slots holding q+k weights (loaded first)
            wo_sb = big.tile([P, D], BF16, tag="wo")
            ident_sb = big.tile([P, P], BF16, tag="ident")
            umask_sb = big.tile([P, P], BF16, tag="umask")

            xh_r = xh.rearrange("(t p) s -> t p s", p=P)
            xl_r = xl.rearrange("(t p) s -> t p s", p=P)
            out_view = out.rearrange("(g p) (j f) -> p g j f", p=P, j=2)

            xt_tiles = {}
            psc_tiles = {}

            def prefetch(sc):
                th = xts.tile([P, NKT, SC], F8, tag="xth", name="xth")
                tl = xts.tile([P, NKT, SC], F8, tag="xtl", name="xtl")
                cols = slice(sc * SC, (sc + 1) * SC)
                steps = [4, 4] if sc <= 1 else [NKT]
                # all of xh before xl: with 1-term q/k the early projection
                # fillers consume only xh; xl is first needed by the v-proj
                k0 = 0
                for st in steps:
                    nc.sync.dma_start(
                        th[:, k0:k0 + st, :],
                        xh_r[k0:k0 + st, :, cols].transpose([1, 0, 2]),
                    )
                    k0 += st
                k0 = 0
                for st in steps:
                    nc.sync.dma_start(
                        tl[:, k0:k0 + st, :],
                        xl_r[k0:k0 + st, :, cols].transpose([1, 0, 2]),
                    )
                    k0 += st
                xt_tiles[sc] = (th, tl)

            # startup DMA order: wqh first (the first matmul's stationary)
            # on the SP ring — the scalar ring's sequencer is blocked ~1.3us
            # by the activation-table load at t=0; wqd and the rest follow
            # on the scalar ring, interleaving with the x pieces
            nc.sync.dma_start(w8_sb[:, 0:1, :, :], w8[:, 0:1, :, :])
            prefetch(0)
            nc.scalar.dma_start(w8_sb[:, 1:NQW, :, :], w8[:, 1:NQW, :, :])
            nc.scalar.dma_start(w8_sb[:, NQW:, :, :], w8[:, NQW:, :, :])
            nc.scalar.dma_start(ident_sb[:], ident[:])
            nc.scalar.dma_start(umask_sb[:], umask[:])
            nc.scalar.dma_start(wo_sb[:], wo[:])
            nc.gpsimd.memset(vone[:, :, 128:130], 1.0)
            # zero the pair-1 slots once; score matmuls read them every tile
            nc.gpsimd.memset(qt[:, 1, :], 0.0)
            nc.gpsimd.memset(kt[:, 1, :], 0.0)

            def proj_fillers(sc, terms, dst, dst_pair=False, post=None, tagc=""):
                """fp8 DoubleRow projection: len(terms)*4 single-matmul
                closures accumulating into one PSUM tile; the last also
                evicts (fp8 pair layout or bf16) and runs post.

                terms: list of (w8 slot, which_x) with which_x 0=xh 1=xl."""
                cols = slice(sc * SC, (sc + 1) * SC)
                box = {}
                n = len(terms) * 4
                # xh-consuming terms first, the xl term last: chunk 0's
                # inline projections then never wait on the xl transfers,
                # which queue behind the weight pieces at startup
                order = ([i for i, (sl, xi) in enumerate(terms) if xi == 0]
                         + [i for i, (sl, xi) in enumerate(terms) if xi == 1])
                seq = [(ti, p) for ti in order for p in range(4)]

                def mk(i):
                    ti, kp = seq[i]
                    wslot, xi = terms[ti]

                    def f():
                        if i == 0:
                            box["ps"] = ps_p.tile([P, SC], F32, tag="pp", name="pp")
                        xt = xt_tiles[sc][xi]
                        _lbl(nc.tensor.matmul(
                            box["ps"][:], w8_sb[:, wslot, 2 * kp:2 * kp + 2, :],
                            xt[:, 2 * kp:2 * kp + 2, :],
                            start=(i == 0), stop=(i == n - 1), perf_mode=DR,
                        ), f"proj{sc}.{tagc}.{i}")
                        if i == n - 1:
                            with nc.allow_low_precision(reason="fp8 q/k evict, tol 2e-2"):
                                if dst_pair:
                                    nc.vector.tensor_copy(dst[:, 0, cols], box["ps"][:])
                                else:
                                    nc.vector.tensor_copy(dst[:, cols], box["ps"][:])
                            if post is not None:
                                post()
                    return f

                return [mk(i) for i in range(n)]

            def vtr_filler(sc):
                """All 4 V-transposes of the chunk into one PSUM tile with a
                single batched vone eviction (4x fewer DVE init rounds, one
                ps_p ring cycle instead of four). Must run as one unit so
                the held ring slot can't wedge interleaved allocations."""
                def f():
                    g0 = sc * 4
                    psT = ps_p.tile([P, 4, P], BF16, tag="pp", name="psT")
                    for gg in range(4):
                        # one accumulation group: the 4 slices share a 2KB
                        # PSUM zero-region, so only the first may start=True
                        # (a later start would re-mark earlier slices as
                        # pending-zero); disjoint slices accumulate onto
                        # zeroed bytes, which is plain writing
                        _lbl(nc.tensor.matmul(
                            psT[:, gg, :],
                            vt[:, (g0 + gg) * P:(g0 + gg + 1) * P],
                            ident_sb[:], is_transpose=True,
                            start=(gg == 0), stop=(gg == 3),
                            skip_group_check=True,
                        ), f"vtr{sc}.{gg}")
                    nc.vector.tensor_copy(vone[:, g0:g0 + 4, 0:128], psT[:])
                return f

            def vp_fillers(sc):
                vps = proj_fillers(sc, VTERMS, vt, tagc="v")
                return vps + [vtr_filler(sc)], []

            def qp_fillers(sc):
                return proj_fillers(sc, QTERMS, qt, dst_pair=True, tagc="q")

            def kp_fillers(sc):
                return proj_fillers(sc, KTERMS, kt, dst_pair=True, tagc="k")

            def oproj_fillers(pc):
                """Out-projection of chunk pc: 8 matmul closures with
                evictions into ob staging alternating DVE/Pool (DVE is the
                contended engine); the last issues the SWDGE store."""
                st0 = pc * 4
                box = {}

                def mk(i):
                    st4, jo = divmod(i, 2)

                    def f():
                        if i == 0:
                            box["ob"] = obs.tile([P, 4, 2, SC], BF16, tag="ob", name="ob")
                        pso = ps_p.tile([P, SC], F32, tag="pp", name="pp")
                        _lbl(nc.tensor.matmul(
                            pso[:], ctxT[:, (st0 + st4) * P:(st0 + st4 + 1) * P],
                            wo_sb[:, jo * SC:(jo + 1) * SC], start=True, stop=True,
                        ), f"oproj{pc}.{i}")
                        # gpsimd cannot read PSUM; ACT takes the evictions
                        # that drain during the exp-light j=0 chunk
                        nc.vector.tensor_copy(box["ob"][:, st4, jo, :], pso[:])
                        if pc >= NSC - 3 and i % 2 == 1:
                            # near the end, store each st-tile as soon as
                            # both halves are evicted — keeps the final DMA
                            # drain short
                            nc.sync.dma_start(
                                out_view[:, st0 + st4:st0 + st4 + 1, :, :],
                                box["ob"][:, st4:st4 + 1, :, :],
                            )
                        elif pc < NSC - 3 and i == 7:
                            nc.sync.dma_start(
                                out_view[:, st0:st0 + 4, :, :], box["ob"][:]
                            )
                    return f

                return [mk(i) for i in range(8)]

            def emit_s(sc, b, j, t, state):
                """Score matmul pair + exp (+ causal select on diag tiles)."""
                nks = 4 * (j + 1)
                g = b * SPB + t
                kcols = slice(g * P, (g + 1) * P)
                diag = t >= nks - 4
                v0 = (t - (nks - 4)) * P if diag else 0
                qw = slice(sc * SC + v0, (sc + 1) * SC)
                pss = ps_sp.tile([P, 2, SC], F32, tag="sc", name="sc")
                for h in range(2):
                    hp = slice(h * 64, (h + 1) * 64)
                    _lbl(nc.tensor.matmul(
                        pss[:, h, v0:], kt[hp, :, kcols], qt[hp, :, qw],
                        start=True, stop=(not diag), perf_mode=DR,
                        tile_position=(h * 64, 0),
                    ), f"score{sc}.t{t}.h{h}")
                    if diag:
                        # causal mask: add -30*2^13 above the diagonal of the
                        # boundary block via ident @ umask; exp then rounds
                        # the masked weights to ~1e-13 (harmless in the AV)
                        _lbl(nc.tensor.matmul(
                            pss[:, h, v0:v0 + P], ident_sb[:], umask_sb[:],
                            start=False, stop=True,
                        ), f"mask{sc}.t{t}.h{h}")
                et = etp.tile([P, 2, SC], BF16, tag="et", name="et")
                _lbl(nc.scalar.activation(et[:, :, v0:], pss[:, :, v0:], EXP,
                                          scale=EXP_SCALE), f"exp{sc}.t{t}")
                state[t] = (et, v0, g)

            def emit_a(sc, j, t, state, psc):
                nks = 4 * (j + 1)
                et, v0, g = state.pop(t)
                if sc == NSC - 1:
                    # last chunk: column halves accumulate into separate
                    # PSUM banks ([65, half, head, 256]) so the left half's
                    # normalize can start before the final AVs land
                    for h in range(2):
                        if v0 < 256:
                            _lbl(nc.tensor.matmul(
                                psc[:, 0, h, v0:], vone[:, g, h:h + 129:2],
                                et[:, h, v0:256],
                                start=(t == 0), stop=(t == nks - 3),
                            ), f"av{sc}.t{t}.h{h}a")
                        b0 = max(v0, 256)
                        _lbl(nc.tensor.matmul(
                            psc[:, 1, h, b0 - 256:], vone[:, g, h:h + 129:2],
                            et[:, h, b0:],
                            start=(t == 0), stop=(t == nks - 1),
                        ), f"av{sc}.t{t}.h{h}b")
                    return
                for h in range(2):
                    _lbl(nc.tensor.matmul(
                        psc[:, h, v0:], vone[:, g, h:h + 129:2], et[:, h, v0:],
                        start=(t == 0), stop=(t == nks - 1),
                    ), f"av{sc}.t{t}.h{h}")

            norm_tmp = {}

            def emit_norm_evict(pc):
                """Evict chunk pc's raw ctx+den PSUM to SBUF (frees the
                single ctx-PSUM slot as early as possible)."""
                psc = psc_tiles[pc]
                tmp = small.tile([65, 2, SC], BF16, tag="tmp", name="tmp")
                nc.vector.tensor_copy(tmp[:], psc[:, :, :])
                norm_tmp[pc] = tmp

            def emit_norm_finish(pc):
                """Normalize chunk pc's evicted ctx into ctxT."""
                ccols = slice(pc * SC, (pc + 1) * SC)
                tmp = norm_tmp.pop(pc)
                for h in range(2):
                    rec = small.tile([1, SC], BF16, tag="rec", name="rec")
                    with nc.allow_low_precision(reason="bf16 softmax denom, tol 2e-2"):
                        nc.vector.reciprocal(rec[:], tmp[64:65, h, :])
                    recb = small.tile([64, SC], BF16, tag="recb", name="recb")
                    nc.gpsimd.partition_broadcast(recb[:], rec[:])
                    rows = slice(h * 64, h * 64 + 64)
                    # all-SBUF multiply on Pool: slower per element but Pool
                    # is the idle engine, and the 2-chunk oproj delay gives
                    # this chain a whole chunk of slack
                    nc.gpsimd.tensor_mul(ctxT[rows, ccols], tmp[0:64, h, :], recb[:])

            def emit_tail_norm_half(pc, half, act_evict=False, pool_muls=False):
                """Last-chunk normalize for one 256-col half: reciprocals
                read the den rows straight from PSUM in parallel with the
                ctx evictions; broadcasts follow on Pool. Returns a closure
                that emits the final muls (defer it so the PE stream isn't
                blocked behind the chain)."""
                psc = psc_tiles[pc]
                base = pc * SC + half * 256
                tmp = small.tile([65, 2, 256], BF16, tag="tmp", name="tmp")
                recbs = []
                with nc.allow_low_precision(reason="bf16 softmax denom, tol 2e-2"):
                    for h in range(2):
                        rec = small.tile([1, 256], BF16, tag="rect", name="rect", bufs=3)
                        nc.vector.reciprocal(rec[:], psc[64:65, half, h, :])
                        recb = small.tile([64, 256], BF16, tag="recbt", name="recbt", bufs=3)
                        nc.gpsimd.partition_broadcast(recb[:], rec[:])
                        recbs.append(recb)
                nc.vector.tensor_copy(tmp[:, 0, :], psc[:, half, 0, :])
                if act_evict:
                    nc.scalar.copy(tmp[:, 1, :], psc[:, half, 1, :])
                else:
                    nc.vector.tensor_copy(tmp[:, 1, :], psc[:, half, 1, :])

                def muls():
                    eng = nc.gpsimd if pool_muls else nc.vector
                    for h in range(2):
                        rows = slice(h * 64, h * 64 + 64)
                        eng.tensor_mul(ctxT[rows, base:base + 256],
                                       tmp[0:64, h, :], recbs[h][:])
                return muls

            def emit_oproj_quarter(pc, qi, split_store=False):
                st = pc * 4 + qi
                ob = obs.tile([P, 1, 2, SC], BF16, tag="obh", name="obh", bufs=4)
                for jo in range(2):
                    pso = ps_p.tile([P, SC], F32, tag="pp", name="pp")
                    _lbl(nc.tensor.matmul(
                        pso[:], ctxT[:, st * P:(st + 1) * P],
                        wo_sb[:, jo * SC:(jo + 1) * SC], start=True, stop=True,
                    ), f"oprojh{pc}.{qi}.{jo}")
                    if qi >= 2 and not (qi == 3 and jo == 1):
                        nc.vector.tensor_copy(ob[:, 0, jo, :], pso[:])
                    else:
                        nc.scalar.copy(ob[:, 0, jo, :], pso[:])
                if split_store:
                    nc.scalar.dma_start(out_view[:, st:st + 1, 0:1, :], ob[:, :, 0, :])
                    nc.sync.dma_start(out_view[:, st:st + 1, 1:2, :], ob[:, :, 1, :])
                else:
                    nc.sync.dma_start(out_view[:, st:st + 1, :, :], ob[:])

            # ---- main pipeline over s-chunks ----
            for sc in range(NSC):
                b, j = divmod(sc, NQC)
                nks = 4 * (j + 1)
                if sc == 0:
                    for f in qp_fillers(0):
                        f()
                    for f in kp_fillers(0):
                        f()
                    vhead0, vtail0 = vp_fillers(0)
                    for f in vhead0:
                        f()
                    prefetch(1)
                    pq0_extra = vtail0
                elif sc + 1 < NSC:
                    prefetch(sc + 1)

                # fillers run during chunk sc: prev chunk's out-proj (deferred
                # one extra chunk near the end so the last chunk stays fed)
                # plus chunk sc+1's projections. The last chunk is the most
                # exp-heavy and has no next-chunk work, so its own k/v
                # projections are held back into its early tiles (legal:
                # they're only consumed by its diagonal tiles, t >= nks-4).
                pq, oq = [], []
                if sc == 0:
                    pq.extend(pq0_extra)
                if sc == NSC - 1:
                    pq.extend(holdover)
                    oq.extend(oproj_fillers(sc - 2))
                    oq.extend(oproj_fillers(sc - 1))
                elif 2 <= sc <= NSC - 2:
                    # out-proj runs TWO chunks after its data: the normalize
                    # chain (recip -> broadcast -> mul) gets a whole chunk of
                    # slack before oproj reads ctxT
                    oq.extend(oproj_fillers(sc - 2))
                if sc + 1 < NSC:
                    vhead, vtail = vp_fillers(sc + 1)
                    qs = qp_fillers(sc + 1)
                    if sc + 1 == NSC - 1:
                        # hold back k-proj and the v-transpose tail for the
                        # last chunk (self-contained PSUM lifetimes only —
                        # holding the v-proj accumulator across the chunk
                        # boundary would wedge the ps_p ring)
                        holdover = vtail + kp_fillers(sc + 1)
                        pq.extend(vhead + qs)
                    else:
                        pq.extend(vhead + qs[0:1] + vtail[0:1] + qs[1:2]
                                  + vtail[1:2] + qs[2:])
                        pq.extend(kp_fillers(sc + 1))

                if sc == NSC - 1:
                    psc = ps_cp.tile([65, 2, 2, 256], F32, tag="ctx", name="ctx")
                else:
                    psc = ps_cp.tile([65, 2, SC], F32, tag="ctx", name="ctx")
                psc_tiles[sc] = psc
                state = {}
                emit_s(sc, b, j, 0, state)
                if sc > 0:
                    emit_norm_finish(sc - 1)
                for t in range(nks):
                    if t + 1 < nks:
                        emit_s(sc, b, j, t + 1, state)
                    diag = t >= nks - 4
                    if diag or sc == 0:
                        # prefer cheap proj fillers inside the diagonal run;
                        # at most one oproj (its eviction is the slow step).
                        # The last chunk pops lazily so fillers remain to
                        # cover its tail-normalize chain.
                        # (last chunk: lighter pops keep the DVE queue clear
                        # for the half-A normalize chain under tiles 13-15)
                        took_op = False
                        for _ in range(6 if sc == 0 else 4):
                            if pq:
                                pq.pop(0)()
                            elif oq and not took_op:
                                oq.pop(0)()
                                took_op = True
                    else:
                        # steady state: ~350ns of filler per tile hides the
                        # exp latency; fp8 proj fillers are ~107ns each.
                        # oproj only after the chunk's normalize (t >= 2).
                        if t >= 2 and oq:
                            oq.pop(0)()
                            if pq:
                                pq.pop(0)()
                        else:
                            for _ in range(3):
                                if pq:
                                    pq.pop(0)()
                    emit_a(sc, j, t, state, psc)
                    if sc == NSC - 1:
                        # half A of the last chunk's ctx is final two tiles
                        # early: run its normalize under the remaining AVs
                        if t == nks - 3:
                            mulsA = emit_tail_norm_half(sc, 0)
                        elif t == nks - 2:
                            mulsA()
                # evict this chunk's ctx PSUM right away: frees the single
                # ctx-PSUM slot before the filler flush queues more DVE work,
                # so the next chunk's first AV doesn't stall on the eviction.
                # The recip/broadcast/mul finish runs early next chunk.
                if sc < NSC - 1:
                    emit_norm_evict(sc)
                    # flush leftovers; three proj pops per oproj pop spaces
                    # the oproj eviction ring
                    while pq or oq:
                        for _ in range(3):
                            if pq:
                                pq.pop(0)()
                        if oq:
                            oq.pop(0)()
                else:
                    # hand-interleaved tail: half A's ctxT is already
                    # normalized (under the last AVs), so quarters 0/1 fire
                    # immediately; half B's chain runs under them and the
                    # banked fillers, then quarters 2/3 finish
                    def pops(n):
                        for _ in range(n):
                            if oq:
                                oq.pop(0)()
                            elif pq:
                                pq.pop(0)()
                    mulsB = emit_tail_norm_half(sc, 1, act_evict=True)
                    emit_oproj_quarter(sc, 0)
                    pops(2)
                    emit_oproj_quarter(sc, 1)
                    pops(2)
                    mulsB()
                    pops(2)
                    emit_oproj_quarter(sc, 2)
                    while pq or oq:
                        (oq or pq).pop(0)()
                    emit_oproj_quarter(sc, 3, split_store=True)



            if DEBUG_DUMP:
                nc.sync.dma_start(dbg_vone[:], vone[:])
                nc.sync.dma_start(dbg_qt[:], qt[:])
                nc.sync.dma_start(dbg_kt[:], kt[:])
                nc.sync.dma_start(dbg_ctxT[:], ctxT[:])

    nc.compile()
    return nc


_NC_CACHE = None


def _get_nc():
    global _NC_CACHE
    if _NC_CACHE is None:
        _NC_CACHE = _build_nc()
    return _NC_CACHE


def kernel(x, w_q, w_k, w_v, w_o, b_o):
    BF = ml_dtypes.bfloat16
    F8N = ml_dtypes.float8_e4m3
    x = np.asarray(x, dtype=np.float32)
    w_q = np.asarray(w_q, dtype=np.float32)
    w_k = np.asarray(w_k, dtype=np.float32)
    w_v = np.asarray(w_v, dtype=np.float32)
    w_o = np.asarray(w_o, dtype=np.float32)
    b_o = np.asarray(b_o, dtype=np.float32)

    xT = np.ascontiguousarray(x.reshape(BS, D).T)
    xh = xT.astype(F8N)
    xl = (16.0 * (xT - xh.astype(np.float32))).astype(F8N)

    def w_layout(w8):
        # [D, DC] -> [P, NKT, DC] with row t*128+p at [p, t]
        return np.ascontiguousarray(w8.reshape(NKT, P, DC).transpose(1, 0, 2))

    def w_split(w):
        # 32x prescale, then hi/lo fp8 split; whd16 pairs with the 16x-scaled
        # x residual so all terms accumulate at the same PSUM scale.
        wp = 32.0 * w
        wh = wp.astype(F8N)
        wl = (wp - wh.astype(np.float32)).astype(F8N)
        whd16 = (wh.astype(np.float32) / 16.0).astype(F8N)
        return (w_layout(wh), w_layout(whd16), w_layout(wl))

    # interleave V head-dims: projection row r holds head r%2, dim r//2, so
    # the plain [128,128] XBAR transpose lands v columns exactly where the AV
    # matmul's strided stationary slice reads them.
    vperm = np.array([(r % 2) * 64 + r // 2 for r in range(DC)])

    nc = _get_nc()
    in_maps = []
    for c in range(NCORES):
        cols = slice(c * DC, (c + 1) * DC)
        qh, qd, ql = w_split(w_q[:, cols])
        kh, kd, kl = w_split(w_k[:, cols])
        vh, vd, vl = w_split(np.ascontiguousarray(w_v[:, cols][:, vperm]))
        if QK_TERMS == 1:
            slots = [qh, kh, vh, vd, vl]
        else:
            slots = [qh, qd, kh, kd, vh, vd, vl]
            if QK_TERMS == 3:
                slots += [ql, kl]
        umask = np.where(np.arange(P)[:, None] > np.arange(P)[None, :],
                         np.float32(-245760.0), np.float32(0.0))
        m = {
            "xh": xh,
            "xl": xl,
            "ident": np.eye(P).astype(BF),
            "umask": umask.astype(BF),
            "w8": np.ascontiguousarray(np.stack(slots, axis=1)),
            "wo": np.ascontiguousarray(w_o[cols, :] / 32.0).astype(BF),
        }
        in_maps.append(m)

    # The first execution of a freshly-jitted 8-core run can return garbage
    # (NaN) through the PJRT donation path; a re-run in the same process is
    # always clean, so retry on NaN as well as on transport errors.
    acc = None
    for attempt in range(4):
        try:
            res = run_bass_kernel_spmd(nc, in_maps, list(range(NCORES)))
        except Exception:
            if attempt == 3:
                raise
            import time
            time.sleep(2.0)
            continue
        acc = res.results[0]["out"].astype(np.float32)
        for c in range(1, NCORES):
            acc = acc + res.results[c]["out"].astype(np.float32)
        if np.isfinite(acc).all():
            break
    acc = acc + b_o[None, :]
    return acc.reshape(B, S, D)



# revision 111
# speedup vs baseline: 1.0059x; 1.0059x over previous
"""Multi-head causal attention (B=2, S=2048, D=1024, H=16) on 8 trn2 cores.

Sharding: tensor-parallel over heads. Each core owns 2 heads: a 128-column
slice of w_q/w_k/w_v and the matching 128-row slice of w_o. Every core
computes a full [B*S, D] partial output in bf16; the host sums the 8 partials
in f32 and adds the bias.

FP8 DoubleRow matmuls (0.5 PE cycles/row, two 128-deep k-tiles contracted
per instruction) carry the projections and the score matmuls; the AV and
out-projection matmuls stay bf16 (1 cycle/row) because attention-weight /
ctx quantization to fp8 would blow the 2e-2 error budget. Precision is held
by hi/lo fp8 decompositions:

  - x is shipped as xh = fp8(x) plus xl = fp8(16*(x - xh)); weights are
    pre-scaled by 32 on the host and split into wh = fp8(32W),
    whd16 = fp8(wh/16) and (v only) wl = fp8(32W - wh). Q/K projections run
    1 term (xh@wh: the score-side fp8 quantization already dominates that
    path); the V projection runs 3 terms (xh@wh + xl@whd16 + xh@wl), making
    v effectively bf16-exact since v feeds the output linearly.
  - Scores: q,k evicted straight from PSUM to fp8 (the score quantization)
    into a zero-padded pair layout [128p, 2, BS] (pair slot 1 is zeros, so a
    K=64 DoubleRow contraction per head computes an exact fp8 dot). The exp
    activation scale absorbs the 32*32 weight prescale (2^-13).
  - The w_o slice is pre-divided by 32 on the host so the 32x-scaled ctx
    cancels; the softmax denominator ratio is scale-invariant.

Measured on-device absmax rel err 1.37e-2 (gate 2e-2).

The per-core schedule is one software-pipelined PE instruction stream over
8 query chunks of 512 rows:

  - Chunk sc's score/exp/AV tiles are woven with "filler" matmuls: the
    out-projection from two chunks back (giving the normalize chain a full
    chunk of slack) plus chunk sc+1's projections and V transposes, so
    nothing in a chunk waits on its own projections. Fillers hide the ~1us
    ACT exp latency per score tile and keep the PE at the full 2.4GHz
    p-state (any >100ns gap halves the PE clock for 3us).
  - Causal masking rides the score PSUM: an ident @ umask matmul adds
    -30*2^13 above the diagonal of each boundary block before exp, so the
    masked weights round to ~1e-13 and the AV path has no Pool dependency.
  - V is transposed [seq, hd] by PE transposes into `vone`, whose
    ones-columns (at 128/129 of a 64B-aligned 160-elem stride) make the AV
    matmul accumulate the softmax denominator for free; host-side
    interleaving of w_v's columns makes the transposed layout match the
    strided (h::2) stationary reads directly. All 4 transposes of a chunk
    share one PSUM tile / accumulation group (start on the first slice
    only: a later start=True would re-mark the shared 2KB zero-region) and
    evict with a single batched DVE copy.
  - Engine placement: PSUM can only be read by DVE/ACT (gpsimd cannot), so
    DVE carries the eviction traffic, ACT carries exp, and Pool gets the
    all-SBUF work (normalize muls, broadcasts, memsets).
  - Chunk normalize is split: the ctx PSUM eviction lands at chunk end
    (freeing the single ctx-PSUM slot before the filler flush), the
    recip/broadcast/mul finish runs early next chunk, and oproj consumes
    ctxT one chunk later still.
  - PSUM budget (8 banks): 2 proj/oproj+transpose ring, 4 score pair ring,
    2 ctx accumulator.
  - The last chunk accumulates ctx column-halves into separate PSUM banks
    ([65, half, head, 256]) so the left half normalizes under the final
    AVs; the right half's chain (PSUM-direct reciprocals in parallel with
    DVE+ACT evictions) runs under banked fillers, and the out-proj
    quarters chase the muls with per-st-tile stores, the last one split
    across two DMA rings.
  - DMA: wqh loads first on the SP ring (the scalar ring's sequencer is
    blocked ~1.3us by the activation-table load), then the first x chunk,
    then the remaining packed-weight pieces.

Cost-model timeline: 113.3us per core (bf16 predecessor: 147.1us).
"""

import sys

sys.path.insert(0, "/opt/trn_rl_repo")

import numpy as np
import ml_dtypes

import concourse.bass as bass
import concourse.mybir as mybir
import concourse.tile as tile
from concourse import bacc
from concourse.bass_utils import run_bass_kernel_spmd

B, S, D, H, HD = 2, 2048, 1024, 16, 64
BS = B * S                  # 4096 flattened rows
NCORES = 8
DC = D // NCORES            # 128 head-dims per core (2 heads)
P = 128                     # partitions
SC = 512                    # s-chunk (moving free dim)
NSC = BS // SC              # 8 s-chunks over the flattened rows
NKT = D // P                # 8 k-tiles for the projections
NQC = S // SC               # 4 q-chunks per batch
NST = BS // P               # 32 s-tiles of 128
SPB = S // P                # 16 s-tiles per batch

F32 = mybir.dt.float32
BF16 = mybir.dt.bfloat16
F8 = mybir.dt.float8e4
DR = mybir.MatmulPerfMode.DoubleRow
DEBUG_DUMP = False
EXP = mybir.ActivationFunctionType.Exp
EXP_SCALE = 0.125 / 1024.0  # 2^-13: 1/sqrt(64) plus the 32*32 weight prescale
QK_TERMS = 1  # hi/lo terms in the q/k projections (1, 2 or 3); 1-term adds
              # only ~1e-3 absmax error (score-side fp8 quantization already
              # dominates the q/k path) and saves 16k PE cycles

LABELS = {}


def _lbl(bi, label):
    try:
        LABELS[bi.ins.name] = label
    except Exception:
        pass
    return bi


def _rr(*groups):
    """Round-robin interleave lists (preserving each list's order)."""
    out = []
    idx = [0] * len(groups)
    while True:
        progressed = False
        for gi, g in enumerate(groups):
            if idx[gi] < len(g):
                out.append(g[idx[gi]])
                idx[gi] += 1
                progressed = True
        if not progressed:
            return out


def _build_nc():
    nc = bacc.Bacc(None, target_bir_lowering=False)

    NW8 = {1: 5, 2: 7, 3: 9}[QK_TERMS]
    xh = nc.dram_tensor("xh", [D, BS], F8, kind="ExternalInput")
    xl = nc.dram_tensor("xl", [D, BS], F8, kind="ExternalInput")
    # all fp8 weight splits packed into one tensor; slot order depends on
    # QK_TERMS (see _w8_slots)
    w8 = nc.dram_tensor("w8", [P, NW8, NKT, DC], F8, kind="ExternalInput")
    wo = nc.dram_tensor("wo", [DC, D], BF16, kind="ExternalInput")
    ident = nc.dram_tensor("ident", [P, P], BF16, kind="ExternalInput")
    # causal mask constant: -245760 (= -30 * 2^13, exp-scale units) above the
    # diagonal, 0 elsewhere; added into the score PSUM via an ident matmul.
    umask = nc.dram_tensor("umask", [P, P], BF16, kind="ExternalInput")
    out = nc.dram_tensor("out", [BS, D], BF16, kind="ExternalOutput")
    if DEBUG_DUMP:
        dbg_vone = nc.dram_tensor("dbg_vone", [P, NST, 160], BF16, kind="ExternalOutput")
        dbg_qt = nc.dram_tensor("dbg_qt", [P, 2, BS], F8, kind="ExternalOutput")
        dbg_kt = nc.dram_tensor("dbg_kt", [P, 2, BS], F8, kind="ExternalOutput")
        dbg_ctxT = nc.dram_tensor("dbg_ctxT", [P, BS], BF16, kind="ExternalOutput")

    with tile.TileContext(nc) as tc:
        with (
            tc.tile_pool(name="big", bufs=1) as big,
            tc.tile_pool(name="xts", bufs=2) as xts,
            tc.tile_pool(name="ob", bufs=4) as obs,
            tc.tile_pool(name="et", bufs=6) as etp,
            tc.tile_pool(name="small", bufs=4) as small,
            tc.tile_pool(name="ps_p", bufs=2, space="PSUM") as ps_p,   # proj + oproj [128,512]
            tc.tile_pool(name="ps_s", bufs=2, space="PSUM") as ps_sp,  # score pairs [128,2,512]
            tc.tile_pool(name="ps_c", bufs=1, space="PSUM") as ps_cp,  # ctx pair [65,2,512]
        ):
            # fp8 q/k in zero-padded pair layout: slot 0 carries the data,
            # slot 1 stays zero so a K=64 DoubleRow matmul contracts exactly.
            qt = big.tile([P, 2, BS], F8, tag="qt")
            kt = big.tile([P, 2, BS], F8, tag="kt")
            vt = big.tile([P, BS], BF16, tag="vt")
            ctxT = big.tile([P, BS], BF16, tag="ctxT")
            vone = big.tile([P, NST, 160], BF16, tag="vone")
            w8_sb = big.tile([P, NW8, NKT, DC], F8, tag="w8")
            # weight-slot (slot, which_x) term lists per projection
            if QK_TERMS == 1:
                QTERMS, KTERMS = [(0, 0)], [(1, 0)]
                VTERMS = [(2, 0), (3, 1), (4, 0)]
            elif QK_TERMS == 2:
                QTERMS, KTERMS = [(0, 0), (1, 1)], [(2, 0), (3, 1)]
                VTERMS = [(4, 0), (5, 1), (6, 0)]
            else:
                QTERMS = [(0, 0), (1, 1), (7, 0)]
                KTERMS = [(2, 0), (3, 1), (8, 0)]
                VTERMS = [(4, 0), (5, 1), (6, 0)]
            NQW = len(QTERMS) * 2  # slots holding q+k weights (loaded first)
            wo_sb = big.tile([P, D], BF16, tag="wo")
            ident_sb = big.tile([P, P], BF16, tag="ident")
            umask_sb = big.tile([P, P], BF16, tag="umask")

            xh_r = xh.rearrange("(t p) s -> t p s", p=P)
            xl_r = xl.rearrange("(t p) s -> t p s", p=P)
            out_view = out.rearrange("(g p) (j f) -> p g j f", p=P, j=2)

            xt_tiles = {}
            psc_tiles = {}

            def prefetch(sc):
                th = xts.tile([P, NKT, SC], F8, tag="xth", name="xth")
                tl = xts.tile([P, NKT, SC], F8, tag="xtl", name="xtl")
                cols = slice(sc * SC, (sc + 1) * SC)
                steps = [4, 4] if sc <= 1 else [NKT]
                # all of xh before xl: with 1-term q/k the early projection
                # fillers consume only xh; xl is first needed by the v-proj
                k0 = 0
                for st in steps:
                    nc.sync.dma_start(
                        th[:, k0:k0 + st, :],
                        xh_r[k0:k0 + st, :, cols].transpose([1, 0, 2]),
                    )
                    k0 += st
                k0 = 0
                for st in steps:
                    nc.sync.dma_start(
                        tl[:, k0:k0 + st, :],
                        xl_r[k0:k0 + st, :, cols].transpose([1, 0, 2]),
                    )
                    k0 += st
                xt_tiles[sc] = (th, tl)

            # startup DMA order: wqh first (the first matmul's stationary)
            # on the SP ring — the scalar ring's sequencer is blocked ~1.3us
            # by the activation-table load at t=0; wqd and the rest follow
            # on the scalar ring, interleaving with the x pieces
            nc.sync.dma_start(w8_sb[:, 0:1, :, :], w8[:, 0:1, :, :])
            prefetch(0)
            nc.scalar.dma_start(w8_sb[:, 1:NQW, :, :], w8[:, 1:NQW, :, :])
            nc.scalar.dma_start(w8_sb[:, NQW:, :, :], w8[:, NQW:, :, :])
            nc.scalar.dma_start(ident_sb[:], ident[:])
            nc.scalar.dma_start(umask_sb[:], umask[:])
            nc.scalar.dma_start(wo_sb[:], wo[:])
            nc.gpsimd.memset(vone[:, :, 128:130], 1.0)
            # zero the pair-1 slots once; score matmuls read them every tile
            nc.gpsimd.memset(qt[:, 1, :], 0.0)
            nc.gpsimd.memset(kt[:, 1, :], 0.0)

            def proj_fillers(sc, terms, dst, dst_pair=False, post=None, tagc=""):
                """fp8 DoubleRow projection: len(terms)*4 single-matmul
                closures accumulating into one PSUM tile; the last also
                evicts (fp8 pair layout or bf16) and runs post.

                terms: list of (w8 slot, which_x) with which_x 0=xh 1=xl."""
                cols = slice(sc * SC, (sc + 1) * SC)
                box = {}
                n = len(terms) * 4
                # xh-consuming terms first, the xl term last: chunk 0's
                # inline projections then never wait on the xl transfers,
                # which queue behind the weight pieces at startup
                order = ([i for i, (sl, xi) in enumerate(terms) if xi == 0]
                         + [i for i, (sl, xi) in enumerate(terms) if xi == 1])
                seq = [(ti, p) for ti in order for p in range(4)]

                def mk(i):
                    ti, kp = seq[i]
                    wslot, xi = terms[ti]

                    def f():
                        if i == 0:
                            box["ps"] = ps_p.tile([P, SC], F32, tag="pp", name="pp")
                        xt = xt_tiles[sc][xi]
                        _lbl(nc.tensor.matmul(
                            box["ps"][:], w8_sb[:, wslot, 2 * kp:2 * kp + 2, :],
                            xt[:, 2 * kp:2 * kp + 2, :],
                            start=(i == 0), stop=(i == n - 1), perf_mode=DR,
                        ), f"proj{sc}.{tagc}.{i}")
                        if i == n - 1:
                            with nc.allow_low_precision(reason="fp8 q/k evict, tol 2e-2"):
                                if dst_pair:
                                    nc.vector.tensor_copy(dst[:, 0, cols], box["ps"][:])
                                else:
                                    nc.vector.tensor_copy(dst[:, cols], box["ps"][:])
                            if post is not None:
                                post()
                    return f

                return [mk(i) for i in range(n)]

            def vtr_filler(sc):
                """All 4 V-transposes of the chunk into one PSUM tile with a
                single batched vone eviction (4x fewer DVE init rounds, one
                ps_p ring cycle instead of four). Must run as one unit so
                the held ring slot can't wedge interleaved allocations."""
                def f():
                    g0 = sc * 4
                    psT = ps_p.tile([P, 4, P], BF16, tag="pp", name="psT")
                    for gg in range(4):
                        # one accumulation group: the 4 slices share a 2KB
                        # PSUM zero-region, so only the first may start=True
                        # (a later start would re-mark earlier slices as
                        # pending-zero); disjoint slices accumulate onto
                        # zeroed bytes, which is plain writing
                        _lbl(nc.tensor.matmul(
                            psT[:, gg, :],
                            vt[:, (g0 + gg) * P:(g0 + gg + 1) * P],
                            ident_sb[:], is_transpose=True,
                            start=(gg == 0), stop=(gg == 3),
                            skip_group_check=True,
                        ), f"vtr{sc}.{gg}")
                    nc.vector.tensor_copy(vone[:, g0:g0 + 4, 0:128], psT[:])
                return f

            def vp_fillers(sc):
                vps = proj_fillers(sc, VTERMS, vt, tagc="v")
                return vps + [vtr_filler(sc)], []

            def qp_fillers(sc):
                return proj_fillers(sc, QTERMS, qt, dst_pair=True, tagc="q")

            def kp_fillers(sc):
                return proj_fillers(sc, KTERMS, kt, dst_pair=True, tagc="k")

            def oproj_fillers(pc):
                """Out-projection of chunk pc: 8 matmul closures with
                evictions into ob staging alternating DVE/Pool (DVE is the
                contended engine); the last issues the SWDGE store."""
                st0 = pc * 4
                box = {}

                def mk(i):
                    st4, jo = divmod(i, 2)

                    def f():
                        if i == 0:
                            box["ob"] = obs.tile([P, 4, 2, SC], BF16, tag="ob", name="ob")
                        pso = ps_p.tile([P, SC], F32, tag="pp", name="pp")
                        _lbl(nc.tensor.matmul(
                            pso[:], ctxT[:, (st0 + st4) * P:(st0 + st4 + 1) * P],
                            wo_sb[:, jo * SC:(jo + 1) * SC], start=True, stop=True,
                        ), f"oproj{pc}.{i}")
                        # gpsimd cannot read PSUM; ACT takes the evictions
                        # that drain during the exp-light j=0 chunk
                        nc.vector.tensor_copy(box["ob"][:, st4, jo, :], pso[:])
                        if pc >= NSC - 3 and i % 2 == 1:
                            # near the end, store each st-tile as soon as
                            # both halves are evicted — keeps the final DMA
                            # drain short
                            nc.sync.dma_start(
                                out_view[:, st0 + st4:st0 + st4 + 1, :, :],
                                box["ob"][:, st4:st4 + 1, :, :],
                            )
                        elif pc < NSC - 3 and i == 7:
                            nc.sync.dma_start(
                                out_view[:, st0:st0 + 4, :, :], box["ob"][:]
                            )
                    return f

                return [mk(i) for i in range(8)]

            def emit_s(sc, b, j, t, state):
                """Score matmul pair + exp (+ causal select on diag tiles)."""
                nks = 4 * (j + 1)
                g = b * SPB + t
                kcols = slice(g * P, (g + 1) * P)
                diag = t >= nks - 4
                v0 = (t - (nks - 4)) * P if diag else 0
                qw = slice(sc * SC + v0, (sc + 1) * SC)
                pss = ps_sp.tile([P, 2, SC], F32, tag="sc", name="sc")
                for h in range(2):
                    hp = slice(h * 64, (h + 1) * 64)
                    _lbl(nc.tensor.matmul(
                        pss[:, h, v0:], kt[hp, :, kcols], qt[hp, :, qw],
                        start=True, stop=(not diag), perf_mode=DR,
                        tile_position=(h * 64, 0),
                    ), f"score{sc}.t{t}.h{h}")
                    if diag:
                        # causal mask: add -30*2^13 above the diagonal of the
                        # boundary block via ident @ umask; exp then rounds
                        # the masked weights to ~1e-13 (harmless in the AV)
                        _lbl(nc.tensor.matmul(
                            pss[:, h, v0:v0 + P], ident_sb[:], umask_sb[:],
                            start=False, stop=True,
                        ), f"mask{sc}.t{t}.h{h}")
                et = etp.tile([P, 2, SC], BF16, tag="et", name="et")
                _lbl(nc.scalar.activation(et[:, :, v0:], pss[:, :, v0:], EXP,
                                          scale=EXP_SCALE), f"exp{sc}.t{t}")
                state[t] = (et, v0, g)

            def emit_a(sc, j, t, state, psc):
                nks = 4 * (j + 1)
                et, v0, g = state.pop(t)
                if sc == NSC - 1:
                    # last chunk: column halves accumulate into separate
                    # PSUM banks ([65, half, head, 256]) so the left half's
                    # normalize can start before the final AVs land
                    for h in range(2):
                        if v0 < 256:
                            _lbl(nc.tensor.matmul(
                                psc[:, 0, h, v0:], vone[:, g, h:h + 129:2],
                                et[:, h, v0:256],
                                start=(t == 0), stop=(t == nks - 3),
                            ), f"av{sc}.t{t}.h{h}a")
                        b0 = max(v0, 256)
                        _lbl(nc.tensor.matmul(
                            psc[:, 1, h, b0 - 256:], vone[:, g, h:h + 129:2],
                            et[:, h, b0:],
                            start=(t == 0), stop=(t == nks - 1),
                        ), f"av{sc}.t{t}.h{h}b")
                    return
                for h in range(2):
                    _lbl(nc.tensor.matmul(
                        psc[:, h, v0:], vone[:, g, h:h + 129:2], et[:, h, v0:],
                        start=(t == 0), stop=(t == nks - 1),
                    ), f"av{sc}.t{t}.h{h}")

            norm_tmp = {}

            def emit_norm_evict(pc):
                """Evict chunk pc's raw ctx+den PSUM to SBUF (frees the
                single ctx-PSUM slot as early as possible)."""
                psc = psc_tiles[pc]
                tmp = small.tile([65, 2, SC], BF16, tag="tmp", name="tmp")
                nc.vector.tensor_copy(tmp[:], psc[:, :, :])
                norm_tmp[pc] = tmp

            def emit_norm_finish(pc):
                """Normalize chunk pc's evicted ctx into ctxT."""
                ccols = slice(pc * SC, (pc + 1) * SC)
                tmp = norm_tmp.pop(pc)
                for h in range(2):
                    rec = small.tile([1, SC], BF16, tag="rec", name="rec")
                    with nc.allow_low_precision(reason="bf16 softmax denom, tol 2e-2"):
                        nc.vector.reciprocal(rec[:], tmp[64:65, h, :])
                    recb = small.tile([64, SC], BF16, tag="recb", name="recb")
                    nc.gpsimd.partition_broadcast(recb[:], rec[:])
                    rows = slice(h * 64, h * 64 + 64)
                    # all-SBUF multiply on Pool: slower per element but Pool
                    # is the idle engine, and the 2-chunk oproj delay gives
                    # this chain a whole chunk of slack
                    nc.gpsimd.tensor_mul(ctxT[rows, ccols], tmp[0:64, h, :], recb[:])

            def emit_tail_norm_half(pc, half, act_evict=False, pool_muls=False):
                """Last-chunk normalize for one 256-col half: reciprocals
                read the den rows straight from PSUM in parallel with the
                ctx evictions; broadcasts follow on Pool. Returns a closure
                that emits the final muls (defer it so the PE stream isn't
                blocked behind the chain)."""
                psc = psc_tiles[pc]
                base = pc * SC + half * 256
                tmp = small.tile([65, 2, 256], BF16, tag="tmp", name="tmp")
                recbs = []
                with nc.allow_low_precision(reason="bf16 softmax denom, tol 2e-2"):
                    for h in range(2):
                        rec = small.tile([1, 256], BF16, tag="rect", name="rect", bufs=3)
                        nc.vector.reciprocal(rec[:], psc[64:65, half, h, :])
                        recb = small.tile([64, 256], BF16, tag="recbt", name="recbt", bufs=3)
                        nc.gpsimd.partition_broadcast(recb[:], rec[:])
                        recbs.append(recb)
                nc.vector.tensor_copy(tmp[:, 0, :], psc[:, half, 0, :])
                if act_evict:
                    nc.scalar.copy(tmp[:, 1, :], psc[:, half, 1, :])
                else:
                    nc.vector.tensor_copy(tmp[:, 1, :], psc[:, half, 1, :])

                def muls():
                    eng = nc.gpsimd if pool_muls else nc.vector
                    for h in range(2):
                        rows = slice(h * 64, h * 64 + 64)
                        eng.tensor_mul(ctxT[rows, base:base + 256],
                                       tmp[0:64, h, :], recbs[h][:])
                return muls

            def emit_oproj_quarter(pc, qi, split_store=False):
                st = pc * 4 + qi
                ob = obs.tile([P, 1, 2, SC], BF16, tag="obh", name="obh", bufs=4)
                for jo in range(2):
                    pso = ps_p.tile([P, SC], F32, tag="pp", name="pp")
                    _lbl(nc.tensor.matmul(
                        pso[:], ctxT[:, st * P:(st + 1) * P],
                        wo_sb[:, jo * SC:(jo + 1) * SC], start=True, stop=True,
                    ), f"oprojh{pc}.{qi}.{jo}")
                    if qi >= 2 and not (qi == 3 and jo == 1):
                        nc.vector.tensor_copy(ob[:, 0, jo, :], pso[:])
                    else:
                        nc.scalar.copy(ob[:, 0, jo, :], pso[:])
                if split_store:
                    nc.scalar.dma_start(out_view[:, st:st + 1, 0:1, :], ob[:, :, 0, :])
                    nc.sync.dma_start(out_view[:, st:st + 1, 1:2, :], ob[:, :, 1, :])
                else:
                    nc.sync.dma_start(out_view[:, st:st + 1, :, :], ob[:])

            # ---- main pipeline over s-chunks ----
            for sc in range(NSC):
                b, j = divmod(sc, NQC)
                nks = 4 * (j + 1)
                if sc == 0:
                    for f in qp_fillers(0):
                        f()
                    for f in kp_fillers(0):
                        f()
                    vhead0, vtail0 = vp_fillers(0)
                    for f in vhead0:
                        f()
                    prefetch(1)
                    pq0_extra = vtail0
                elif sc + 1 < NSC:
                    prefetch(sc + 1)

                # fillers run during chunk sc: prev chunk's out-proj (deferred
                # one extra chunk near the end so the last chunk stays fed)
                # plus chunk sc+1's projections. The last chunk is the most
                # exp-heavy and has no next-chunk work, so its own k/v
                # projections are held back into its early tiles (legal:
                # they're only consumed by its diagonal tiles, t >= nks-4).
                pq, oq = [], []
                if sc == 0:
                    pq.extend(pq0_extra)
                if sc == NSC - 1:
                    pq.extend(holdover)
                    oq.extend(oproj_fillers(sc - 2))
                    oq.extend(oproj_fillers(sc - 1))
                elif 2 <= sc <= NSC - 2:
                    # out-proj runs TWO chunks after its data: the normalize
                    # chain (recip -> broadcast -> mul) gets a whole chunk of
                    # slack before oproj reads ctxT
                    oq.extend(oproj_fillers(sc - 2))
                if sc + 1 < NSC:
                    vhead, vtail = vp_fillers(sc + 1)
                    qs = qp_fillers(sc + 1)
                    if sc + 1 == NSC - 1:
                        # hold back k-proj and the v-transpose tail for the
                        # last chunk (self-contained PSUM lifetimes only —
                        # holding the v-proj accumulator across the chunk
                        # boundary would wedge the ps_p ring)
                        holdover = vtail + kp_fillers(sc + 1)
                        pq.extend(vhead + qs)
                    else:
                        pq.extend(vhead + qs[0:1] + vtail[0:1] + qs[1:2]
                                  + vtail[1:2] + qs[2:])
                        pq.extend(kp_fillers(sc + 1))

                if sc == NSC - 1:
                    psc = ps_cp.tile([65, 2, 2, 256], F32, tag="ctx", name="ctx")
                else:
                    psc = ps_cp.tile([65, 2, SC], F32, tag="ctx", name="ctx")
                psc_tiles[sc] = psc
                state = {}
                emit_s(sc, b, j, 0, state)
                for t in range(nks):
                    if t + 1 < nks:
                        emit_s(sc, b, j, t + 1, state)
                    diag = t >= nks - 4
                    if diag or sc == 0:
                        # prefer cheap proj fillers inside the diagonal run;
                        # at most one oproj (its eviction is the slow step).
                        # The last chunk pops lazily so fillers remain to
                        # cover its tail-normalize chain.
                        # (last chunk: lighter pops keep the DVE queue clear
                        # for the half-A normalize chain under tiles 13-15)
                        took_op = False
                        for _ in range(6 if sc == 0 else 4):
                            if pq:
                                pq.pop(0)()
                            elif oq and not took_op:
                                oq.pop(0)()
                                took_op = True
                    else:
                        # steady state: ~350ns of filler per tile hides the
                        # exp latency; fp8 proj fillers are ~107ns each.
                        # oproj only after the chunk's normalize (t >= 2).
                        if t >= 2 and oq:
                            oq.pop(0)()
                            if pq:
                                pq.pop(0)()
                        else:
                            for _ in range(3):
                                if pq:
                                    pq.pop(0)()
                    emit_a(sc, j, t, state, psc)
                    if sc > 0 and t == nks // 2:
                        # mid-chunk: the DVE queue is clear of the chunk-start
                        # projection evictions, and oproj(sc-1) doesn't read
                        # ctxT until next chunk — a full chunk of slack
                        emit_norm_finish(sc - 1)
                    if sc == NSC - 1:
                        # half A of the last chunk's ctx is final two tiles
                        # early: run its normalize under the remaining AVs
                        if t == nks - 3:
                            mulsA = emit_tail_norm_half(sc, 0)
                        elif t == nks - 2:
                            mulsA()
                # evict this chunk's ctx PSUM right away: frees the single
                # ctx-PSUM slot before the filler flush queues more DVE work,
                # so the next chunk's first AV doesn't stall on the eviction.
                # The recip/broadcast/mul finish runs early next chunk.
                if sc < NSC - 1:
                    emit_norm_evict(sc)
                    # flush leftovers; three proj pops per oproj pop spaces
                    # the oproj eviction ring
                    while pq or oq:
                        for _ in range(3):
                            if pq:
                                pq.pop(0)()
                        if oq:
                            oq.pop(0)()
                else:
                    # hand-interleaved tail: half A's ctxT is already
                    # normalized (under the last AVs), so quarters 0/1 fire
                    # immediately; half B's chain runs under them and the
                    # banked fillers, then quarters 2/3 finish
                    def pops(n):
                        for _ in range(n):
                            if oq:
                                oq.pop(0)()
                            elif pq:
                                pq.pop(0)()
                    mulsB = emit_tail_norm_half(sc, 1, act_evict=True)
                    emit_oproj_quarter(sc, 0)
                    pops(2)
                    emit_oproj_quarter(sc, 1)
                    pops(2)
                    mulsB()
                    pops(2)
                    emit_oproj_quarter(sc, 2)
                    while pq or oq:
                        (oq or pq).pop(0)()
                    emit_oproj_quarter(sc, 3, split_store=True)



            if DEBUG_DUMP:
                nc.sync.dma_start(dbg_vone[:], vone[:])
                nc.sync.dma_start(dbg_qt[:], qt[:])
                nc.sync.dma_start(dbg_kt[:], kt[:])
                nc.sync.dma_start(dbg_ctxT[:], ctxT[:])

    nc.compile()
    return nc


_NC_CACHE = None


def _get_nc():
    global _NC_CACHE
    if _NC_CACHE is None:
        _NC_CACHE = _build_nc()
    return _NC_CACHE


def kernel(x, w_q, w_k, w_v, w_o, b_o):
    BF = ml_dtypes.bfloat16
    F8N = ml_dtypes.float8_e4m3
    x = np.asarray(x, dtype=np.float32)
    w_q = np.asarray(w_q, dtype=np.float32)
    w_k = np.asarray(w_k, dtype=np.float32)
    w_v = np.asarray(w_v, dtype=np.float32)
    w_o = np.asarray(w_o, dtype=np.float32)
    b_o = np.asarray(b_o, dtype=np.float32)

    xT = np.ascontiguousarray(x.reshape(BS, D).T)
    xh = xT.astype(F8N)
    xl = (16.0 * (xT - xh.astype(np.float32))).astype(F8N)

    def w_layout(w8):
        # [D, DC] -> [P, NKT, DC] with row t*128+p at [p, t]
        return np.ascontiguousarray(w8.reshape(NKT, P, DC).transpose(1, 0, 2))

    def w_split(w):
        # 32x prescale, then hi/lo fp8 split; whd16 pairs with the 16x-scaled
        # x residual so all terms accumulate at the same PSUM scale.
        wp = 32.0 * w
        wh = wp.astype(F8N)
        wl = (wp - wh.astype(np.float32)).astype(F8N)
        whd16 = (wh.astype(np.float32) / 16.0).astype(F8N)
        return (w_layout(wh), w_layout(whd16), w_layout(wl))

    # interleave V head-dims: projection row r holds head r%2, dim r//2, so
    # the plain [128,128] XBAR transpose lands v columns exactly where the AV
    # matmul's strided stationary slice reads them.
    vperm = np.array([(r % 2) * 64 + r // 2 for r in range(DC)])

    nc = _get_nc()
    in_maps = []
    for c in range(NCORES):
        cols = slice(c * DC, (c + 1) * DC)
        qh, qd, ql = w_split(w_q[:, cols])
        kh, kd, kl = w_split(w_k[:, cols])
        vh, vd, vl = w_split(np.ascontiguousarray(w_v[:, cols][:, vperm]))
        if QK_TERMS == 1:
            slots = [qh, kh, vh, vd, vl]
        else:
            slots = [qh, qd, kh, kd, vh, vd, vl]
            if QK_TERMS == 3:
                slots += [ql, kl]
        umask = np.where(np.arange(P)[:, None] > np.arange(P)[None, :],
                         np.float32(-245760.0), np.float32(0.0))
        m = {
            "xh": xh,
            "xl": xl,
            "ident": np.eye(P).astype(BF),
            "umask": umask.astype(BF),
            "w8": np.ascontiguousarray(np.stack(slots, axis=1)),
            "wo": np.ascontiguousarray(w_o[cols, :] / 32.0).astype(BF),
        }
        in_maps.append(m)

    # The first execution of a freshly-jitted 8-core run can return garbage
    # (NaN) through the PJRT donation path; a re-run in the same process is
    # always clean, so retry on NaN as well as on transport errors.
    acc = None
    for attempt in range(4):
        try:
            res = run_bass_kernel_spmd(nc, in_maps, list(range(NCORES)))
        except Exception:
            if attempt == 3:
                raise
            import time
            time.sleep(2.0)
            continue
        acc = res.results[0]["out"].astype(np.float32)
        for c in range(1, NCORES):
            acc = acc + res.results[c]["out"].astype(np.float32)
        if np.isfinite(acc).all():
            break
    acc = acc + b_o[None, :]
    return acc.reshape(B, S, D)



# revision 113
# speedup vs baseline: 1.0101x; 1.0042x over previous
"""Multi-head causal attention (B=2, S=2048, D=1024, H=16) on 8 trn2 cores.

Sharding: tensor-parallel over heads. Each core owns 2 heads: a 128-column
slice of w_q/w_k/w_v and the matching 128-row slice of w_o. Every core
computes a full [B*S, D] partial output in bf16; the host sums the 8 partials
in f32 and adds the bias.

FP8 DoubleRow matmuls (0.5 PE cycles/row, two 128-deep k-tiles contracted
per instruction) carry the projections and the score matmuls; the AV and
out-projection matmuls stay bf16 (1 cycle/row) because attention-weight /
ctx quantization to fp8 would blow the 2e-2 error budget. Precision is held
by hi/lo fp8 decompositions:

  - x is shipped as xh = fp8(x) plus xl = fp8(16*(x - xh)); weights are
    pre-scaled by 32 on the host and split into wh = fp8(32W),
    whd16 = fp8(wh/16) and (v only) wl = fp8(32W - wh). Q/K projections run
    1 term (xh@wh: the score-side fp8 quantization already dominates that
    path); the V projection runs 3 terms (xh@wh + xl@whd16 + xh@wl), making
    v effectively bf16-exact since v feeds the output linearly.
  - Scores: q,k evicted straight from PSUM to fp8 (the score quantization)
    into a zero-padded pair layout [128p, 2, BS] (pair slot 1 is zeros, so a
    K=64 DoubleRow contraction per head computes an exact fp8 dot). The exp
    activation scale absorbs the 32*32 weight prescale (2^-13).
  - The w_o slice is pre-divided by 32 on the host so the 32x-scaled ctx
    cancels; the softmax denominator ratio is scale-invariant.

Measured on-device absmax rel err 1.37e-2 (gate 2e-2).

The per-core schedule is one software-pipelined PE instruction stream over
8 query chunks of 512 rows:

  - Chunk sc's score/exp/AV tiles are woven with "filler" matmuls: the
    out-projection from two chunks back (giving the normalize chain a full
    chunk of slack) plus chunk sc+1's projections and V transposes, so
    nothing in a chunk waits on its own projections. Fillers hide the ~1us
    ACT exp latency per score tile and keep the PE at the full 2.4GHz
    p-state (any >100ns gap halves the PE clock for 3us).
  - Causal masking rides the score PSUM: an ident @ umask matmul adds
    -30*2^13 above the diagonal of each boundary block before exp, so the
    masked weights round to ~1e-13 and the AV path has no Pool dependency.
  - V is transposed [seq, hd] by PE transposes into `vone`, whose
    ones-columns (at 128/129 of a 64B-aligned 160-elem stride) make the AV
    matmul accumulate the softmax denominator for free; host-side
    interleaving of w_v's columns makes the transposed layout match the
    strided (h::2) stationary reads directly. All 4 transposes of a chunk
    share one PSUM tile / accumulation group (start on the first slice
    only: a later start=True would re-mark the shared 2KB zero-region) and
    evict with a single batched DVE copy.
  - Engine placement: PSUM can only be read by DVE/ACT (gpsimd cannot), so
    DVE carries the eviction traffic, ACT carries exp, and Pool gets the
    all-SBUF work (normalize muls, broadcasts, memsets).
  - Chunk normalize is split: the ctx PSUM eviction lands at chunk end
    (freeing the single ctx-PSUM slot before the filler flush), the
    recip/broadcast/mul finish runs MID next chunk (clear of the
    chunk-start projection evictions on the DVE queue), and oproj
    consumes ctxT one chunk later still.
  - PSUM budget (8 banks): 2 proj/oproj+transpose ring, 4 score pair ring,
    2 ctx accumulator.
  - The last chunk accumulates ctx column-halves into separate PSUM banks
    ([65, half, head, 256]) so the left half normalizes under the final
    AVs; the right half's chain (PSUM-direct reciprocals in parallel with
    DVE+ACT evictions) runs under banked fillers, and the out-proj
    quarters chase the muls with per-st-tile stores, the last one split
    across two DMA rings.
  - DMA: wqh loads first on the SP ring (the scalar ring's sequencer is
    blocked ~1.3us by the activation-table load), then the first x chunk,
    then the remaining packed-weight pieces.

Cost-model timeline: 112.6us per core (bf16 predecessor: 147.1us).
"""

import sys

sys.path.insert(0, "/opt/trn_rl_repo")

import numpy as np
import ml_dtypes

import concourse.bass as bass
import concourse.mybir as mybir
import concourse.tile as tile
from concourse import bacc
from concourse.bass_utils import run_bass_kernel_spmd

B, S, D, H, HD = 2, 2048, 1024, 16, 64
BS = B * S                  # 4096 flattened rows
NCORES = 8
DC = D // NCORES            # 128 head-dims per core (2 heads)
P = 128                     # partitions
SC = 512                    # s-chunk (moving free dim)
NSC = BS // SC              # 8 s-chunks over the flattened rows
NKT = D // P                # 8 k-tiles for the projections
NQC = S // SC               # 4 q-chunks per batch
NST = BS // P               # 32 s-tiles of 128
SPB = S // P                # 16 s-tiles per batch

F32 = mybir.dt.float32
BF16 = mybir.dt.bfloat16
F8 = mybir.dt.float8e4
DR = mybir.MatmulPerfMode.DoubleRow
DEBUG_DUMP = False
EXP = mybir.ActivationFunctionType.Exp
EXP_SCALE = 0.125 / 1024.0  # 2^-13: 1/sqrt(64) plus the 32*32 weight prescale
QK_TERMS = 1  # hi/lo terms in the q/k projections (1, 2 or 3); 1-term adds
              # only ~1e-3 absmax error (score-side fp8 quantization already
              # dominates the q/k path) and saves 16k PE cycles

LABELS = {}


def _lbl(bi, label):
    try:
        LABELS[bi.ins.name] = label
    except Exception:
        pass
    return bi


def _rr(*groups):
    """Round-robin interleave lists (preserving each list's order)."""
    out = []
    idx = [0] * len(groups)
    while True:
        progressed = False
        for gi, g in enumerate(groups):
            if idx[gi] < len(g):
                out.append(g[idx[gi]])
                idx[gi] += 1
                progressed = True
        if not progressed:
            return out


def _build_nc():
    nc = bacc.Bacc(None, target_bir_lowering=False)

    NW8 = {1: 5, 2: 7, 3: 9}[QK_TERMS]
    xh = nc.dram_tensor("xh", [D, BS], F8, kind="ExternalInput")
    xl = nc.dram_tensor("xl", [D, BS], F8, kind="ExternalInput")
    # all fp8 weight splits packed into one tensor; slot order depends on
    # QK_TERMS (see _w8_slots)
    w8 = nc.dram_tensor("w8", [P, NW8, NKT, DC], F8, kind="ExternalInput")
    wo = nc.dram_tensor("wo", [DC, D], BF16, kind="ExternalInput")
    ident = nc.dram_tensor("ident", [P, P], BF16, kind="ExternalInput")
    # causal mask constant: -245760 (= -30 * 2^13, exp-scale units) above the
    # diagonal, 0 elsewhere; added into the score PSUM via an ident matmul.
    umask = nc.dram_tensor("umask", [P, P], BF16, kind="ExternalInput")
    out = nc.dram_tensor("out", [BS, D], BF16, kind="ExternalOutput")
    if DEBUG_DUMP:
        dbg_vone = nc.dram_tensor("dbg_vone", [P, NST, 160], BF16, kind="ExternalOutput")
        dbg_qt = nc.dram_tensor("dbg_qt", [P, 2, BS], F8, kind="ExternalOutput")
        dbg_kt = nc.dram_tensor("dbg_kt", [P, 2, BS], F8, kind="ExternalOutput")
        dbg_ctxT = nc.dram_tensor("dbg_ctxT", [P, BS], BF16, kind="ExternalOutput")

    with tile.TileContext(nc) as tc:
        with (
            tc.tile_pool(name="big", bufs=1) as big,
            tc.tile_pool(name="xts", bufs=2) as xts,
            tc.tile_pool(name="ob", bufs=4) as obs,
            tc.tile_pool(name="et", bufs=6) as etp,
            tc.tile_pool(name="small", bufs=4) as small,
            tc.tile_pool(name="ps_p", bufs=2, space="PSUM") as ps_p,   # proj + oproj [128,512]
            tc.tile_pool(name="ps_s", bufs=2, space="PSUM") as ps_sp,  # score pairs [128,2,512]
            tc.tile_pool(name="ps_c", bufs=1, space="PSUM") as ps_cp,  # ctx pair [65,2,512]
        ):
            # fp8 q/k in zero-padded pair layout: slot 0 carries the data,
            # slot 1 stays zero so a K=64 DoubleRow matmul contracts exactly.
            qt = big.tile([P, 2, BS], F8, tag="qt")
            kt = big.tile([P, 2, BS], F8, tag="kt")
            vt = big.tile([P, BS], BF16, tag="vt")
            ctxT = big.tile([P, BS], BF16, tag="ctxT")
            vone = big.tile([P, NST, 160], BF16, tag="vone")
            w8_sb = big.tile([P, NW8, NKT, DC], F8, tag="w8")
            # weight-slot (slot, which_x) term lists per projection
            if QK_TERMS == 1:
                QTERMS, KTERMS = [(0, 0)], [(1, 0)]
                VTERMS = [(2, 0), (3, 1), (4, 0)]
            elif QK_TERMS == 2:
                QTERMS, KTERMS = [(0, 0), (1, 1)], [(2, 0), (3, 1)]
                VTERMS = [(4, 0), (5, 1), (6, 0)]
            else:
                QTERMS = [(0, 0), (1, 1), (7, 0)]
                KTERMS = [(2, 0), (3, 1), (8, 0)]
                VTERMS = [(4, 0), (5, 1), (6, 0)]
            NQW = len(QTERMS) * 2  # slots holding q+k weights (loaded first)
            wo_sb = big.tile([P, D], BF16, tag="wo")
            ident_sb = big.tile([P, P], BF16, tag="ident")
            umask_sb = big.tile([P, P], BF16, tag="umask")

            xh_r = xh.rearrange("(t p) s -> t p s", p=P)
            xl_r = xl.rearrange("(t p) s -> t p s", p=P)
            out_view = out.rearrange("(g p) (j f) -> p g j f", p=P, j=2)

            xt_tiles = {}
            psc_tiles = {}

            def prefetch(sc):
                th = xts.tile([P, NKT, SC], F8, tag="xth", name="xth")
                tl = xts.tile([P, NKT, SC], F8, tag="xtl", name="xtl")
                cols = slice(sc * SC, (sc + 1) * SC)
                steps = [4, 4] if sc <= 1 else [NKT]
                # all of xh before xl: with 1-term q/k the early projection
                # fillers consume only xh; xl is first needed by the v-proj
                k0 = 0
                for st in steps:
                    nc.sync.dma_start(
                        th[:, k0:k0 + st, :],
                        xh_r[k0:k0 + st, :, cols].transpose([1, 0, 2]),
                    )
                    k0 += st
                k0 = 0
                for st in steps:
                    nc.sync.dma_start(
                        tl[:, k0:k0 + st, :],
                        xl_r[k0:k0 + st, :, cols].transpose([1, 0, 2]),
                    )
                    k0 += st
                xt_tiles[sc] = (th, tl)

            # startup DMA order: wqh first (the first matmul's stationary)
            # on the SP ring — the scalar ring's sequencer is blocked ~1.3us
            # by the activation-table load at t=0; wqd and the rest follow
            # on the scalar ring, interleaving with the x pieces
            nc.sync.dma_start(w8_sb[:, 0:1, :, :], w8[:, 0:1, :, :])
            prefetch(0)
            nc.scalar.dma_start(w8_sb[:, 1:NQW, :, :], w8[:, 1:NQW, :, :])
            nc.scalar.dma_start(w8_sb[:, NQW:, :, :], w8[:, NQW:, :, :])
            nc.scalar.dma_start(ident_sb[:], ident[:])
            nc.scalar.dma_start(umask_sb[:], umask[:])
            nc.scalar.dma_start(wo_sb[:], wo[:])
            nc.gpsimd.memset(vone[:, :, 128:130], 1.0)
            # zero the pair-1 slots once; score matmuls read them every tile
            nc.gpsimd.memset(qt[:, 1, :], 0.0)
            nc.gpsimd.memset(kt[:, 1, :], 0.0)

            def proj_fillers(sc, terms, dst, dst_pair=False, post=None, tagc=""):
                """fp8 DoubleRow projection: len(terms)*4 single-matmul
                closures accumulating into one PSUM tile; the last also
                evicts (fp8 pair layout or bf16) and runs post.

                terms: list of (w8 slot, which_x) with which_x 0=xh 1=xl."""
                cols = slice(sc * SC, (sc + 1) * SC)
                box = {}
                n = len(terms) * 4
                # xh-consuming terms first, the xl term last: chunk 0's
                # inline projections then never wait on the xl transfers,
                # which queue behind the weight pieces at startup
                order = ([i for i, (sl, xi) in enumerate(terms) if xi == 0]
                         + [i for i, (sl, xi) in enumerate(terms) if xi == 1])
                seq = [(ti, p) for ti in order for p in range(4)]

                def mk(i):
                    ti, kp = seq[i]
                    wslot, xi = terms[ti]

                    def f():
                        if i == 0:
                            box["ps"] = ps_p.tile([P, SC], F32, tag="pp", name="pp")
                        xt = xt_tiles[sc][xi]
                        _lbl(nc.tensor.matmul(
                            box["ps"][:], w8_sb[:, wslot, 2 * kp:2 * kp + 2, :],
                            xt[:, 2 * kp:2 * kp + 2, :],
                            start=(i == 0), stop=(i == n - 1), perf_mode=DR,
                        ), f"proj{sc}.{tagc}.{i}")
                        if i == n - 1:
                            with nc.allow_low_precision(reason="fp8 q/k evict, tol 2e-2"):
                                if dst_pair:
                                    nc.vector.tensor_copy(dst[:, 0, cols], box["ps"][:])
                                else:
                                    nc.vector.tensor_copy(dst[:, cols], box["ps"][:])
                            if post is not None:
                                post()
                    return f

                return [mk(i) for i in range(n)]

            def vtr_filler(sc):
                """All 4 V-transposes of the chunk into one PSUM tile with a
                single batched vone eviction (4x fewer DVE init rounds, one
                ps_p ring cycle instead of four). Must run as one unit so
                the held ring slot can't wedge interleaved allocations."""
                def f():
                    g0 = sc * 4
                    psT = ps_p.tile([P, 4, P], BF16, tag="pp", name="psT")
                    for gg in range(4):
                        # one accumulation group: the 4 slices share a 2KB
                        # PSUM zero-region, so only the first may start=True
                        # (a later start would re-mark earlier slices as
                        # pending-zero); disjoint slices accumulate onto
                        # zeroed bytes, which is plain writing
                        _lbl(nc.tensor.matmul(
                            psT[:, gg, :],
                            vt[:, (g0 + gg) * P:(g0 + gg + 1) * P],
                            ident_sb[:], is_transpose=True,
                            start=(gg == 0), stop=(gg == 3),
                            skip_group_check=True,
                        ), f"vtr{sc}.{gg}")
                    nc.vector.tensor_copy(vone[:, g0:g0 + 4, 0:128], psT[:])
                return f

            def vp_fillers(sc):
                vps = proj_fillers(sc, VTERMS, vt, tagc="v")
                return vps + [vtr_filler(sc)], []

            def qp_fillers(sc):
                return proj_fillers(sc, QTERMS, qt, dst_pair=True, tagc="q")

            def kp_fillers(sc):
                return proj_fillers(sc, KTERMS, kt, dst_pair=True, tagc="k")

            def oproj_fillers(pc):
                """Out-projection of chunk pc: 8 matmul closures with
                evictions into ob staging alternating DVE/Pool (DVE is the
                contended engine); the last issues the SWDGE store."""
                st0 = pc * 4
                box = {}

                def mk(i):
                    st4, jo = divmod(i, 2)

                    def f():
                        if i == 0:
                            box["ob"] = obs.tile([P, 4, 2, SC], BF16, tag="ob", name="ob")
                        pso = ps_p.tile([P, SC], F32, tag="pp", name="pp")
                        _lbl(nc.tensor.matmul(
                            pso[:], ctxT[:, (st0 + st4) * P:(st0 + st4 + 1) * P],
                            wo_sb[:, jo * SC:(jo + 1) * SC], start=True, stop=True,
                        ), f"oproj{pc}.{i}")
                        # gpsimd cannot read PSUM; ACT takes the evictions
                        # that drain during the exp-light j=0 chunk
                        nc.vector.tensor_copy(box["ob"][:, st4, jo, :], pso[:])
                        if pc >= NSC - 3 and i % 2 == 1:
                            # near the end, store each st-tile as soon as
                            # both halves are evicted — keeps the final DMA
                            # drain short
                            nc.sync.dma_start(
                                out_view[:, st0 + st4:st0 + st4 + 1, :, :],
                                box["ob"][:, st4:st4 + 1, :, :],
                            )
                        elif pc < NSC - 3 and i == 7:
                            nc.sync.dma_start(
                                out_view[:, st0:st0 + 4, :, :], box["ob"][:]
                            )
                    return f

                return [mk(i) for i in range(8)]

            def emit_s(sc, b, j, t, state):
                """Score matmul pair + exp (+ causal select on diag tiles)."""
                nks = 4 * (j + 1)
                g = b * SPB + t
                kcols = slice(g * P, (g + 1) * P)
                diag = t >= nks - 4
                v0 = (t - (nks - 4)) * P if diag else 0
                qw = slice(sc * SC + v0, (sc + 1) * SC)
                pss = ps_sp.tile([P, 2, SC], F32, tag="sc", name="sc")
                for h in range(2):
                    hp = slice(h * 64, (h + 1) * 64)
                    _lbl(nc.tensor.matmul(
                        pss[:, h, v0:], kt[hp, :, kcols], qt[hp, :, qw],
                        start=True, stop=(not diag), perf_mode=DR,
                        tile_position=(h * 64, 0),
                    ), f"score{sc}.t{t}.h{h}")
                    if diag:
                        # causal mask: add -30*2^13 above the diagonal of the
                        # boundary block via ident @ umask; exp then rounds
                        # the masked weights to ~1e-13 (harmless in the AV)
                        _lbl(nc.tensor.matmul(
                            pss[:, h, v0:v0 + P], ident_sb[:], umask_sb[:],
                            start=False, stop=True,
                        ), f"mask{sc}.t{t}.h{h}")
                et = etp.tile([P, 2, SC], BF16, tag="et", name="et")
                _lbl(nc.scalar.activation(et[:, :, v0:], pss[:, :, v0:], EXP,
                                          scale=EXP_SCALE), f"exp{sc}.t{t}")
                state[t] = (et, v0, g)

            def emit_a(sc, j, t, state, psc):
                nks = 4 * (j + 1)
                et, v0, g = state.pop(t)
                if sc == NSC - 1:
                    # last chunk: column halves accumulate into separate
                    # PSUM banks ([65, half, head, 256]) so the left half's
                    # normalize can start before the final AVs land
                    for h in range(2):
                        if v0 < 256:
                            _lbl(nc.tensor.matmul(
                                psc[:, 0, h, v0:], vone[:, g, h:h + 129:2],
                                et[:, h, v0:256],
                                start=(t == 0), stop=(t == nks - 3),
                            ), f"av{sc}.t{t}.h{h}a")
                        b0 = max(v0, 256)
                        _lbl(nc.tensor.matmul(
                            psc[:, 1, h, b0 - 256:], vone[:, g, h:h + 129:2],
                            et[:, h, b0:],
                            start=(t == 0), stop=(t == nks - 1),
                        ), f"av{sc}.t{t}.h{h}b")
                    return
                for h in range(2):
                    _lbl(nc.tensor.matmul(
                        psc[:, h, v0:], vone[:, g, h:h + 129:2], et[:, h, v0:],
                        start=(t == 0), stop=(t == nks - 1),
                    ), f"av{sc}.t{t}.h{h}")

            norm_tmp = {}

            def emit_norm_evict(pc):
                """Evict chunk pc's raw ctx+den PSUM to SBUF (frees the
                single ctx-PSUM slot as early as possible)."""
                psc = psc_tiles[pc]
                tmp = small.tile([65, 2, SC], BF16, tag="tmp", name="tmp")
                nc.vector.tensor_copy(tmp[:], psc[:, :, :])
                norm_tmp[pc] = tmp

            def emit_norm_finish(pc, h_only=None):
                """Normalize chunk pc's evicted ctx into ctxT (optionally
                one head at a time, to halve the DVE burst)."""
                ccols = slice(pc * SC, (pc + 1) * SC)
                if h_only is None or h_only == 1:
                    tmp = norm_tmp.pop(pc)
                else:
                    tmp = norm_tmp[pc]
                heads = range(2) if h_only is None else [h_only]
                for h in heads:
                    rec = small.tile([1, SC], BF16, tag="rec", name="rec")
                    with nc.allow_low_precision(reason="bf16 softmax denom, tol 2e-2"):
                        nc.vector.reciprocal(rec[:], tmp[64:65, h, :])
                    recb = small.tile([64, SC], BF16, tag="recb", name="recb")
                    nc.gpsimd.partition_broadcast(recb[:], rec[:])
                    rows = slice(h * 64, h * 64 + 64)
                    # all-SBUF multiply on Pool: slower per element but Pool
                    # is the idle engine, and the 2-chunk oproj delay gives
                    # this chain a whole chunk of slack
                    nc.gpsimd.tensor_mul(ctxT[rows, ccols], tmp[0:64, h, :], recb[:])

            def emit_tail_norm_half(pc, half, act_evict=False, pool_muls=False):
                """Last-chunk normalize for one 256-col half: reciprocals
                read the den rows straight from PSUM in parallel with the
                ctx evictions; broadcasts follow on Pool. Returns a closure
                that emits the final muls (defer it so the PE stream isn't
                blocked behind the chain)."""
                psc = psc_tiles[pc]
                base = pc * SC + half * 256
                tmp = small.tile([65, 2, 256], BF16, tag="tmp", name="tmp")
                recbs = []
                with nc.allow_low_precision(reason="bf16 softmax denom, tol 2e-2"):
                    for h in range(2):
                        rec = small.tile([1, 256], BF16, tag="rect", name="rect", bufs=3)
                        nc.vector.reciprocal(rec[:], psc[64:65, half, h, :])
                        recb = small.tile([64, 256], BF16, tag="recbt", name="recbt", bufs=3)
                        nc.gpsimd.partition_broadcast(recb[:], rec[:])
                        recbs.append(recb)
                nc.vector.tensor_copy(tmp[:, 0, :], psc[:, half, 0, :])
                if act_evict:
                    nc.scalar.copy(tmp[:, 1, :], psc[:, half, 1, :])
                else:
                    nc.vector.tensor_copy(tmp[:, 1, :], psc[:, half, 1, :])

                def muls():
                    eng = nc.gpsimd if pool_muls else nc.vector
                    for h in range(2):
                        rows = slice(h * 64, h * 64 + 64)
                        eng.tensor_mul(ctxT[rows, base:base + 256],
                                       tmp[0:64, h, :], recbs[h][:])
                return muls

            def emit_oproj_quarter(pc, qi, split_store=False):
                st = pc * 4 + qi
                ob = obs.tile([P, 1, 2, SC], BF16, tag="obh", name="obh", bufs=4)
                for jo in range(2):
                    pso = ps_p.tile([P, SC], F32, tag="pp", name="pp")
                    _lbl(nc.tensor.matmul(
                        pso[:], ctxT[:, st * P:(st + 1) * P],
                        wo_sb[:, jo * SC:(jo + 1) * SC], start=True, stop=True,
                    ), f"oprojh{pc}.{qi}.{jo}")
                    if qi >= 2 and not (qi == 3 and jo == 1):
                        nc.vector.tensor_copy(ob[:, 0, jo, :], pso[:])
                    else:
                        nc.scalar.copy(ob[:, 0, jo, :], pso[:])
                if split_store:
                    nc.scalar.dma_start(out_view[:, st:st + 1, 0:1, :], ob[:, :, 0, :])
                    nc.sync.dma_start(out_view[:, st:st + 1, 1:2, :], ob[:, :, 1, :])
                else:
                    nc.sync.dma_start(out_view[:, st:st + 1, :, :], ob[:])

            # ---- main pipeline over s-chunks ----
            for sc in range(NSC):
                b, j = divmod(sc, NQC)
                nks = 4 * (j + 1)
                if sc == 0:
                    for f in qp_fillers(0):
                        f()
                    for f in kp_fillers(0):
                        f()
                    vhead0, vtail0 = vp_fillers(0)
                    for f in vhead0:
                        f()
                    prefetch(1)
                    pq0_extra = vtail0
                elif sc + 1 < NSC:
                    prefetch(sc + 1)

                # fillers run during chunk sc: prev chunk's out-proj (deferred
                # one extra chunk near the end so the last chunk stays fed)
                # plus chunk sc+1's projections. The last chunk is the most
                # exp-heavy and has no next-chunk work, so its own k/v
                # projections are held back into its early tiles (legal:
                # they're only consumed by its diagonal tiles, t >= nks-4).
                pq, oq = [], []
                if sc == 0:
                    pq.extend(pq0_extra)
                if sc == NSC - 1:
                    pq.extend(holdover)
                    oq.extend(oproj_fillers(sc - 2))
                    oq.extend(oproj_fillers(sc - 1))
                elif 2 <= sc <= NSC - 2:
                    # out-proj runs TWO chunks after its data: the normalize
                    # chain (recip -> broadcast -> mul) gets a whole chunk of
                    # slack before oproj reads ctxT
                    oq.extend(oproj_fillers(sc - 2))
                if sc + 1 < NSC:
                    vhead, vtail = vp_fillers(sc + 1)
                    qs = qp_fillers(sc + 1)
                    if sc + 1 == NSC - 1:
                        # hold back k-proj and the v-transpose tail for the
                        # last chunk (self-contained PSUM lifetimes only —
                        # holding the v-proj accumulator across the chunk
                        # boundary would wedge the ps_p ring)
                        holdover = vtail + kp_fillers(sc + 1)
                        pq.extend(vhead + qs)
                    else:
                        pq.extend(vhead + qs[0:1] + vtail[0:1] + qs[1:2]
                                  + vtail[1:2] + qs[2:])
                        pq.extend(kp_fillers(sc + 1))

                if sc == NSC - 1:
                    psc = ps_cp.tile([65, 2, 2, 256], F32, tag="ctx", name="ctx")
                else:
                    psc = ps_cp.tile([65, 2, SC], F32, tag="ctx", name="ctx")
                psc_tiles[sc] = psc
                state = {}
                emit_s(sc, b, j, 0, state)
                for t in range(nks):
                    if t + 1 < nks:
                        emit_s(sc, b, j, t + 1, state)
                    diag = t >= nks - 4
                    if diag or sc == 0:
                        # prefer cheap proj fillers inside the diagonal run;
                        # at most one oproj (its eviction is the slow step).
                        # The last chunk pops lazily so fillers remain to
                        # cover its tail-normalize chain.
                        # (last chunk: lighter pops keep the DVE queue clear
                        # for the half-A normalize chain under tiles 13-15)
                        took_op = False
                        for _ in range(6 if sc == 0 else 4):
                            if pq:
                                pq.pop(0)()
                            elif oq and not took_op:
                                oq.pop(0)()
                                took_op = True
                    else:
                        # steady state: ~350ns of filler per tile hides the
                        # exp latency; fp8 proj fillers are ~107ns each.
                        # oproj only after the chunk's normalize (t >= 2).
                        if t >= 2 and oq:
                            oq.pop(0)()
                            if pq:
                                pq.pop(0)()
                        else:
                            for _ in range(3):
                                if pq:
                                    pq.pop(0)()
                    emit_a(sc, j, t, state, psc)
                    if sc > 0 and t == nks // 2:
                        # mid-chunk: the DVE queue is clear of the chunk-start
                        # projection evictions, and oproj(sc-1) doesn't read
                        # ctxT until next chunk — a full chunk of slack;
                        # per-head halves keep the DVE burst under the ring
                        # stall threshold
                        emit_norm_finish(sc - 1, h_only=0)
                    elif sc > 0 and t == nks // 2 + 1:
                        emit_norm_finish(sc - 1, h_only=1)
                    if sc == NSC - 1:
                        # half A of the last chunk's ctx is final two tiles
                        # early: run its normalize under the remaining AVs
                        if t == nks - 3:
                            mulsA = emit_tail_norm_half(sc, 0)
                        elif t == nks - 2:
                            mulsA()
                # evict this chunk's ctx PSUM right away: frees the single
                # ctx-PSUM slot before the filler flush queues more DVE work,
                # so the next chunk's first AV doesn't stall on the eviction.
                # The recip/broadcast/mul finish runs early next chunk.
                if sc < NSC - 1:
                    emit_norm_evict(sc)
                    # flush leftovers; three proj pops per oproj pop spaces
                    # the oproj eviction ring
                    while pq or oq:
                        for _ in range(3):
                            if pq:
                                pq.pop(0)()
                        if oq:
                            oq.pop(0)()
                else:
                    # hand-interleaved tail: half A's ctxT is already
                    # normalized (under the last AVs), so quarters 0/1 fire
                    # immediately; half B's chain runs under them and the
                    # banked fillers, then quarters 2/3 finish
                    def pops(n):
                        for _ in range(n):
                            if oq:
                                oq.pop(0)()
                            elif pq:
                                pq.pop(0)()
                    mulsB = emit_tail_norm_half(sc, 1, act_evict=True)
                    emit_oproj_quarter(sc, 0)
                    pops(2)
                    emit_oproj_quarter(sc, 1)
                    pops(2)
                    mulsB()
                    pops(2)
                    emit_oproj_quarter(sc, 2)
                    while pq or oq:
                        (oq or pq).pop(0)()
                    emit_oproj_quarter(sc, 3, split_store=True)



            if DEBUG_DUMP:
                nc.sync.dma_start(dbg_vone[:], vone[:])
                nc.sync.dma_start(dbg_qt[:], qt[:])
                nc.sync.dma_start(dbg_kt[:], kt[:])
                nc.sync.dma_start(dbg_ctxT[:], ctxT[:])

    nc.compile()
    return nc


_NC_CACHE = None


def _get_nc():
    global _NC_CACHE
    if _NC_CACHE is None:
        _NC_CACHE = _build_nc()
    return _NC_CACHE


def kernel(x, w_q, w_k, w_v, w_o, b_o):
    BF = ml_dtypes.bfloat16
    F8N = ml_dtypes.float8_e4m3
    x = np.asarray(x, dtype=np.float32)
    w_q = np.asarray(w_q, dtype=np.float32)
    w_k = np.asarray(w_k, dtype=np.float32)
    w_v = np.asarray(w_v, dtype=np.float32)
    w_o = np.asarray(w_o, dtype=np.float32)
    b_o = np.asarray(b_o, dtype=np.float32)

    xT = np.ascontiguousarray(x.reshape(BS, D).T)
    xh = xT.astype(F8N)
    xl = (16.0 * (xT - xh.astype(np.float32))).astype(F8N)

    def w_layout(w8):
        # [D, DC] -> [P, NKT, DC] with row t*128+p at [p, t]
        return np.ascontiguousarray(w8.reshape(NKT, P, DC).transpose(1, 0, 2))

    def w_split(w):
        # 32x prescale, then hi/lo fp8 split; whd16 pairs with the 16x-scaled
        # x residual so all terms accumulate at the same PSUM scale.
        wp = 32.0 * w
        wh = wp.astype(F8N)
        wl = (wp - wh.astype(np.float32)).astype(F8N)
        whd16 = (wh.astype(np.float32) / 16.0).astype(F8N)
        return (w_layout(wh), w_layout(whd16), w_layout(wl))

    # interleave V head-dims: projection row r holds head r%2, dim r//2, so
    # the plain [128,128] XBAR transpose lands v columns exactly where the AV
    # matmul's strided stationary slice reads them.
    vperm = np.array([(r % 2) * 64 + r // 2 for r in range(DC)])

    nc = _get_nc()
    in_maps = []
    for c in range(NCORES):
        cols = slice(c * DC, (c + 1) * DC)
        qh, qd, ql = w_split(w_q[:, cols])
        kh, kd, kl = w_split(w_k[:, cols])
        vh, vd, vl = w_split(np.ascontiguousarray(w_v[:, cols][:, vperm]))
        if QK_TERMS == 1:
            slots = [qh, kh, vh, vd, vl]
        else:
            slots = [qh, qd, kh, kd, vh, vd, vl]
            if QK_TERMS == 3:
                slots += [ql, kl]
        umask = np.where(np.arange(P)[:, None] > np.arange(P)[None, :],
                         np.float32(-245760.0), np.float32(0.0))
        m = {
            "xh": xh,
            "xl": xl,
            "ident": np.eye(P).astype(BF),
            "umask": umask.astype(BF),
            "w8": np.ascontiguousarray(np.stack(slots, axis=1)),
            "wo": np.ascontiguousarray(w_o[cols, :] / 32.0).astype(BF),
        }
        in_maps.append(m)

    # The first execution of a freshly-jitted 8-core run can return garbage
    # (NaN) through the PJRT donation path; a re-run in the same process is
    # always clean, so retry on NaN as well as on transport errors.
    acc = None
    for attempt in range(4):
        try:
            res = run_bass_kernel_spmd(nc, in_maps, list(range(NCORES)))
        except Exception:
            if attempt == 3:
                raise
            import time
            time.sleep(2.0)
            continue
        acc = res.results[0]["out"].astype(np.float32)
        for c in range(1, NCORES):
            acc = acc + res.results[c]["out"].astype(np.float32)
        if np.isfinite(acc).all():
            break
    acc = acc + b_o[None, :]
    return acc.reshape(B, S, D)

